# revision 8
# baseline (speedup 1.0000x reference)
"""Raw-Bass bf16 MoE kernel (v3) — minimal instruction count.

This environment executes ~1 instruction per ~35-70us regardless of content
(measured; see micro.py), so the kernel is designed to minimize the number of
EXECUTED instructions:
  - bf16 matmuls (measured ~25-40% cheaper than f32r; rel err ~4e-3 << 2e-2)
  - no Tile framework: semaphore waits/updates are attached directly to the
    instructions that need them (zero extra sync instructions, except one
    NoOp carrier per expert for the double-wait case)
  - all aux work in the fewest, widest ops possible

Dataflow per core (data-parallel over tokens, TOK=1024 per core):
  gate:    logitsT[E,TOK] = Wg^T-stationary matmuls; +bg; exp (ACT)
           transpose exp -> token layout; sum_E; recip -> r_tok
           gtok = exp_tok * r_tok  (normalized gate, [128,TT,E])
  bias:    pb[i] = exp @ be  (PE);  acc[i] = pb[i] * r_tok[i]  (DVE, normalized)
  experts: per (e,i): 16 bf16 matmuls -> pm pair; TSP: acc[i] += pm * gtok[i,e]
  store:   one DMA of acc.
"""
from contextlib import ExitStack

import numpy as np

import concourse.bass as bass
import concourse.mybir as mybir

N_TOKENS, D_IN, D_OUT, E = 8192, 1024, 1024, 8
NCORES = 8
TOK = N_TOKENS // NCORES
P = 128
KT = D_IN // P    # 8 contraction tiles
TT = TOK // P     # 8 token tiles
FH = 512

_F32 = mybir.dt.float32
_BF16 = mybir.dt.bfloat16


def build_v5(reps: int = 1, internal_io: bool = False) -> bass.Bass:
    nc = bass.Bass()
    kind_in = {} if internal_io else {"kind": "ExternalInput"}
    xT_d = nc.dram_tensor("xT", [D_IN, TOK], _BF16, **kind_in)
    We_d = nc.dram_tensor("We", [E, D_IN, D_OUT], _BF16, **kind_in)
    be_d = nc.dram_tensor("be", [E, D_OUT], _BF16, **kind_in)
    Wg_d = nc.dram_tensor("Wg", [D_IN, E], _BF16, **kind_in)
    bg_d = nc.dram_tensor("bg", [E], _F32, **kind_in)
    id_d = nc.dram_tensor("ident", [E, E], _BF16, **kind_in)
    if internal_io:
        out_d = nc.dram_tensor("out", [TOK, D_OUT], _F32)
        probe_d = nc.dram_tensor("probe", [P, P], _F32, kind="ExternalOutput")
    else:
        out_d = nc.dram_tensor("out", [TOK, D_OUT], _F32, kind="ExternalOutput")
        probe_d = None

    ctx = ExitStack()
    # SBUF ([partition, ...]; bf16 unless noted)
    xT = ctx.enter_context(nc.sbuf_tensor("xTs", [P, KT, TOK], _BF16))
    we = ctx.enter_context(nc.sbuf_tensor("wes", [P, 2, 4, KT, D_OUT], _BF16))
    acc = ctx.enter_context(nc.sbuf_tensor("accs", [P, TT, D_OUT], _F32))
    wg = ctx.enter_context(nc.sbuf_tensor("wgs", [P, KT, E], _BF16))
    bgc = ctx.enter_context(nc.sbuf_tensor("bgc", [E, 1], _F32))
    bes = ctx.enter_context(nc.sbuf_tensor("bes", [E, D_OUT], _BF16))
    ident = ctx.enter_context(nc.sbuf_tensor("idents", [E, E], _BF16))
    ltT = ctx.enter_context(nc.sbuf_tensor("ltT", [E, TOK], _F32))
    expT = ctx.enter_context(nc.sbuf_tensor("expT", [E, TOK], _BF16))
    exptok = ctx.enter_context(nc.sbuf_tensor("exptok", [P, TT, E], _F32))
    stok = ctx.enter_context(nc.sbuf_tensor("stok", [P, TT, 1], _F32))
    rtok = ctx.enter_context(nc.sbuf_tensor("rtok", [P, TT, 1], _F32))
    gtok = ctx.enter_context(nc.sbuf_tensor("gtok", [P, TT, E], _F32))
    if internal_io:
        seedf = ctx.enter_context(nc.sbuf_tensor("seedf", [P, D_OUT], _F32))
        seedb = ctx.enter_context(nc.sbuf_tensor("seedb", [P, D_OUT], _BF16))
    # PSUM: 4 pairs of banks as one tensor [128, 4, 1024] f32 (all 8 banks)
    pm = ctx.enter_context(nc.psum_tensor("pm", [P, 4, 1024], _F32))
    # gate logits view [E, 1024] on pair 0; transpose staging on pair 1
    pg = pm[0:E, 0, :]
    ptr = pm[:, 1, 0:32].bitcast(_BF16)  # [128, 64] bf16 in bank 2

    # Semaphores. DMA completions are UNORDERED across in-flight DMAs, so
    # each dependency group gets its own semaphore; a waiter's threshold is
    # only ever satisfied by the exact DMAs it needs.
    semSU = nc.alloc_semaphore("semSU")    # setup + seed DMAs
    semX = nc.alloc_semaphore("semX")      # xT loads (1/rep)
    semW = [nc.alloc_semaphore("semW0"), nc.alloc_semaphore("semW1")]
    semPE = nc.alloc_semaphore("semPE")    # expert-chain completions
    semPEg = nc.alloc_semaphore("semPEg")  # gate/tr/bias PE milestones
    semDVE = nc.alloc_semaphore("semDVE")  # DVE op completions

    su = 0     # semSU cumulative
    pe = 0     # semPE cumulative (expert chain ends)
    peg = 0    # semPEg cumulative
    dve = 0    # semDVE cumulative

    def dma(dst, src, sem, val, wait=None):
        inst = nc.sync.dma_start(dst, src)
        if wait is not None:
            inst.wait_op(wait[0], wait[1], "sem-ge")
        inst.then_inc(sem, 16)
        return val + 16

    def dma_su(dst, src, wait=None):
        nonlocal su
        su = dma(dst, src, semSU, su, wait=wait)
        return su

    if internal_io:
        nc.vector.memset(seedf[:, :], 0.005)
        nc.vector.memset(seedb[:, :], 0.005)
        nc.vector.memset(seedf[:, :], 0.005).then_inc(semDVE, 1)
        dve += 1

        def rep_src(n_rep):
            s = seedb[:, :].opt()
            return bass.AP(tensor=s.tensor, offset=s.offset,
                           ap=[[s.ap[0][0], P], [0, n_rep], [1, D_OUT]])

        # seeds wait on the memsets via semDVE; later SP DMAs dispatch
        # in sequencer order, so only the first needs the wait
        dma_su(xT_d.rearrange("(k p) n -> p k n", p=P), rep_src(KT),
               wait=(semDVE, dve))
        for e in range(E):
            dma_su(We_d[e].rearrange("(k p) o -> p k o", p=P), rep_src(KT))
        dma_su(be_d[:, :], seedb[0:E, :])
        dma_su(Wg_d.rearrange("(k p) e -> p k e", p=P),
               seedb[:, 0:KT * E].rearrange("p (k e) -> p k e", k=KT))
        dma_su(bg_d[:], seedf[0, 0:E])
        dma_su(id_d[:, :], seedb[0:E, 0:E])

    # ---- setup loads (once) ----
    dma_su(wg[:, :, :], Wg_d.rearrange("(k p) e -> p k e", p=P))
    dma_su(bgc[:, :], bg_d[:])
    dma_su(bes[:, :], be_d[:, :])
    dma_su(ident[:, :], id_d[:, :])
    setup_su = su

    last_tsp_dve = None   # semDVE value of final TSP of previous rep
    xv = 0                # semX cumulative
    wv = [0, 0]           # semW slot cumulative

    for r in range(reps):
        # xT load; WAR on xT + all psum banks proven free via last rep's
        # TSPs. In rep 0 the free wait slot instead covers setup/seeds.
        xt_wait = ((semDVE, last_tsp_dve) if last_tsp_dve is not None
                   else (semSU, setup_su))
        xv = dma(xT[:, :, :], xT_d.rearrange("(k p) n -> p k n", p=P),
                 semX, xv, wait=xt_wait)

        slab_val = []
        for sl in range(2):
            # WAR on slab sl: its experts' chains of the previous rep done
            need = 64 * (r - 1) + (4 * sl + 4) * 8
            wait = (semPE, need) if need > 0 else (semSU, setup_su)
            wv[sl] = dma(we[:, sl, :, :, :],
                         We_d[4 * sl:4 * sl + 4].rearrange(
                             "e (k p) o -> p e k o", p=P),
                         semW[sl], wv[sl], wait=wait)
            slab_val.append(wv[sl])

        # ---- gate logits (PE): pg[E, TOK] = sum_k wg[k].T @ xT[k] ----
        for k in range(KT):
            for h in range(TOK // FH):
                inst = nc.tensor.matmul(
                    pg[:, h * FH:(h + 1) * FH], wg[:, k, :],
                    xT[:, k, h * FH:(h + 1) * FH],
                    start=(k == 0), stop=(k == KT - 1))
                if k == 0 and h == 0:
                    # xT (and transitively all setup DMAs) loaded
                    inst.wait_op(semX, xv, "sem-ge")
                if k == KT - 1 and h == TOK // FH - 1:
                    inst.then_inc(semPEg, 1)
        peg += 1
        gate_peg = peg

        # ---- DVE: ltT = pg + bg (per-partition scalar) ----
        inst = nc.vector.tensor_scalar_add(ltT[:, :], pg, bgc[:, :])
        inst.wait_op(semPEg, gate_peg, "sem-ge")
        inst.then_inc(semDVE, 1)
        dve += 1
        tsa_dve = dve

        # ---- ACT: expT = exp(ltT), bf16 out ----
        inst = nc.scalar.activation(expT[:, :], ltT[:, :],
                                    mybir.ActivationFunctionType.Exp)
        inst.wait_op(semDVE, tsa_dve, "sem-ge")
        inst.then_inc(semPEg, 1)  # reuse semPEg lane for ACT->PE handoff
        peg += 1
        exp_peg = peg

        # ---- PE: transpose expT into token-layout staging (bank 2) ----
        for i in range(TT):
            inst = nc.tensor.transpose(ptr[:, i * E:(i + 1) * E],
                                       expT[:, i * P:(i + 1) * P],
                                       ident[:, :])
            if i == 0:
                inst.wait_op(semPEg, exp_peg, "sem-ge")
            if i == TT - 1:
                inst.then_inc(semPEg, 1)
        peg += 1
        tr_peg = peg

        # ---- bias matmuls (PE): pb[i] = exp_block[i].T-stationary @ be ----
        # pb uses pairs 2,3 (banks 4-7), rotating per i; mm(i) must wait for
        # the DVE bias-init of i-2 before clobbering its pair. DVE incs this
        # rep: tsa (dve), copy/reduce/recip/mul (dve+1..4), bias-init(j)
        # (dve+5+j), so bias-init(i-2) completes at semDVE == dve + 3 + i.
        for i in range(TT):
            pb = pm[:, 2 + (i % 2), :]
            for h in range(2):
                inst = nc.tensor.matmul(pb[:, h * FH:(h + 1) * FH],
                                        expT[:, i * P:(i + 1) * P],
                                        bes[:, h * FH:(h + 1) * FH],
                                        start=True, stop=True)
                if h == 0 and i >= 2:
                    inst.wait_op(semDVE, dve + 3 + i, "sem-ge")
                if h == 1:
                    inst.then_inc(semPEg, 1)
                    if i == TT - 1:
                        # slab-0 guard for experts 0-3 (wait slot is free)
                        inst.wait_op(semW[0], slab_val[0], "sem-ge")
                        bias_guard_inst = inst
            peg += 1

        # ---- DVE chain: exp_tok copy, sum, recip, gtok ----
        # Same-engine RAW also needs sem sync on this HW: each DVE op incs
        # semDVE and the next dependent one waits on that value. A wait on a
        # later semDVE value transitively covers all earlier DVE writes and
        # (because the store also incs semDVE) the previous rep's store of
        # acc.
        base = dve
        copy_v, reduce_v, recip_v, mul_v = base + 1, base + 2, base + 3, base + 4
        # write exptok through its canonical 3D AP (the race detector treats
        # reshaped write-views as separate shadow regions); reshape the
        # source instead.
        p3 = ptr[:, :].opt()
        ptr3 = bass.AP(tensor=p3.tensor, offset=p3.offset,
                       ap=[p3.ap[0], [E, TT], [1, E]])
        inst = nc.vector.tensor_copy(exptok[:, :, :], ptr3)
        inst.wait_op(semPEg, tr_peg, "sem-ge")
        inst.then_inc(semDVE, 1)
        inst = nc.vector.reduce_sum(stok[:, :, :], exptok[:, :, :],
                                    axis=mybir.AxisListType.X)
        inst.wait_op(semDVE, copy_v, "sem-ge")
        inst.then_inc(semDVE, 1)
        inst = nc.vector.reciprocal(rtok[:, :, :], stok[:, :, :])
        inst.wait_op(semDVE, reduce_v, "sem-ge")
        inst.then_inc(semDVE, 1)
        r_ap = rtok[:, :, 0:1].opt()
        rb = bass.AP(tensor=r_ap.tensor, offset=r_ap.offset,
                     ap=[r_ap.ap[0], r_ap.ap[1], [0, E]])
        inst = nc.vector.tensor_mul(gtok[:, :, :], exptok[:, :, :], rb)
        inst.wait_op(semDVE, recip_v, "sem-ge")
        inst.then_inc(semDVE, 1)
        dve = mul_v

        # ---- DVE: acc[i] = pb[i] * r_tok[i]  (normalized bias init) ----
        for i in range(TT):
            pb = pm[:, 2 + (i % 2), :]
            inst = nc.vector.tensor_scalar_mul(acc[:, i, :], pb,
                                               rtok[:, i, 0:1])
            inst.wait_op(semPEg, tr_peg + 1 + i, "sem-ge")
            inst.then_inc(semDVE, 1)
            dve += 1
        bias_init_done_dve = dve

        # ---- experts ----
        # tile t = e*TT + i (within rep); psum pair = t % 4; 4-deep
        # pipeline. Slab-guard waits ride existing PE instructions whose
        # wait slot is free: slab 0 on the last bias matmul (i=7, h=1),
        # slab 1 on the last matmul of expert 3 (both carry only incs).
        tsp_dve_of_tile = {}
        for e in range(E):
            for i in range(TT):
                t = e * TT + i
                pair = pm[:, t % 4, :]
                isl = slice(i * P, (i + 1) * P)
                first_wait = None
                if t >= 4:
                    first_wait = (semDVE, tsp_dve_of_tile[t - 4])
                elif e == 0:
                    # pairs 0,1 freed by gate/tr consumers; pairs 2,3 by
                    # bias inits. Conservative single wait: all bias inits.
                    first_wait = (semDVE, bias_init_done_dve)
                for k in range(KT):
                    for h in range(2):
                        inst = nc.tensor.matmul(
                            pair[:, h * FH:(h + 1) * FH],
                            xT[:, k, isl],
                            we[:, e // 4, e % 4, k, h * FH:(h + 1) * FH],
                            start=(k == 0), stop=(k == KT - 1))
                        if k == 0 and h == 0 and first_wait is not None:
                            inst.wait_op(first_wait[0], first_wait[1],
                                         "sem-ge")
                        if k == KT - 1 and h == 1:
                            inst.then_inc(semPE, 1)
                            if e == 3 and i == TT - 1:
                                # slab-1 guard for experts 4-7
                                inst.wait_op(semW[1], slab_val[1], "sem-ge")
                pe += 1
                # TSP on DVE: acc[i] += pm * gtok[i, e]
                inst = nc.vector.scalar_tensor_tensor(
                    out=acc[:, i, :], in0=pair, scalar=gtok[:, i, e:e + 1],
                    in1=acc[:, i, :],
                    op0=mybir.AluOpType.mult, op1=mybir.AluOpType.add)
                inst.wait_op(semPE, pe, "sem-ge")
                inst.then_inc(semDVE, 1)
                dve += 1
                tsp_dve_of_tile[t] = dve

        last_tsp_dve = dve

        # ---- store ----
        inst = nc.sync.dma_start(out_d.rearrange("(i p) o -> p i o", p=P),
                                 acc[:, :, :])
        inst.wait_op(semDVE, last_tsp_dve, "sem-ge")
        inst.then_inc(semDVE, 16)
        dve += 16

    if internal_io:
        inst = nc.sync.dma_start(probe_d[:, :], acc[:, 0, 0:P])
        inst.wait_op(semDVE, dve, "sem-ge")
        inst.then_inc(semDVE, 16)
        dve += 16
    # final quiesce so the NEFF doesn't retire before the stores complete
    nc.sync.wait_ge(semDVE, dve)

    ctx.close()
    return nc


def make_in_maps_v5(x, We, be, Wg, bg):
    import ml_dtypes

    bf = ml_dtypes.bfloat16
    We_c = np.ascontiguousarray(We, dtype=bf)
    be_c = np.ascontiguousarray(be, dtype=bf)
    Wg_c = np.ascontiguousarray(Wg, dtype=bf)
    bg_c = np.ascontiguousarray(bg, dtype=np.float32)
    id_c = np.eye(E, dtype=bf)
    in_maps = []
    for c in range(NCORES):
        xs = np.asarray(x[c * TOK:(c + 1) * TOK], dtype=bf)
        in_maps.append({
            "xT": np.ascontiguousarray(xs.T),
            "We": We_c,
            "be": be_c,
            "Wg": Wg_c,
            "bg": bg_c,
            "ident": id_c,
        })
    return in_maps


_NC_CACHE = {}


def kernel(x, We, be, Wg, bg):
    from concourse.bass_utils import run_bass_kernel_spmd

    if "v5" not in _NC_CACHE:
        _NC_CACHE["v5"] = build_v5()
    nc = _NC_CACHE["v5"]
    in_maps = make_in_maps_v5(x, We, be, Wg, bg)
    res = run_bass_kernel_spmd(nc, in_maps, list(range(NCORES)))
    out = np.concatenate([res.results[c]["out"] for c in range(NCORES)],
                         axis=0)
    return out.astype(np.float32)


# alias for test.py's measure_hw_time
build_v3 = build_v5
